# revision 9
# baseline (speedup 1.0000x reference)
"""AttentionBlock (GroupNorm + single-head self-attention + proj + residual)
on 8 Trainium2 NeuronCores.

Sharding: batch (4) x query-token-half (2) -> 8 shards. Each core gets the
full image of its batch element (for GroupNorm stats and K/V over all 4096
tokens) plus its half of the query tokens; K/V/GN are computed redundantly
by the 2 cores sharing a batch element, which is far cheaper than
cross-core collectives at this size.

Math per core (c=256 channels, n=4096 tokens, nq=2048 query tokens):
  GroupNorm is folded into the QKV weights: xn = s_c * x + t_c with
  per-channel s,t computed on-device from group stats, so
  Q = (wq*s) @ x + (wq@t + bq), etc. The score scale 1/sqrt(c) is folded
  into wk/bk on the host. The V-path bias is folded into the output
  projection bias (b* = wp@(wv@t+bv) + bp).
  Scores are computed k-major: S^T[m,i] = sum_o K[o,m] Q[o,i] so softmax
  needs a cross-partition denominator, obtained by accumulating exp tiles
  on DVE and one all-ones matmul (which also broadcasts the sums to all
  partitions); A@V uses lhsT = V^T (computed directly as x^T @ wv') so no
  transposes are needed anywhere. The attention loop processes all 1024
  query columns of a half-shard at once (2-bank PSUM tiles) to halve the
  ACT/DVE instruction count.

All matmuls run in float32r (TF32-like: fp32 with 11-bit mantissa, full
fp32 accumulate) which streams near bf16 rate -- measured ~2e-4 relative
error on the full block, ~15x better than bf16.
"""

import numpy as np

B, C, H, W = 4, 256, 64, 64
N = H * W            # 4096 tokens
NQ = N // 2          # 2048 query tokens per core
GROUPS = 8
GSIZE = C // GROUPS  # 32 channels per group
EPS = 1e-5
P = 128              # partitions
CC = C // P          # 2 channel chunks
NCORES = 8
QW = 1024            # query columns processed per attention pass
NQP = NQ // QW       # 2 passes

_cache = {}


def round_tf32(x: np.ndarray) -> np.ndarray:
    """Round fp32 to fp32r (11-bit mantissa, round-to-nearest-even)."""
    i = np.ascontiguousarray(x, dtype=np.float32).view(np.uint32)
    r = (i + np.uint32(0x7FF) + ((i >> np.uint32(12)) & np.uint32(1))) & np.uint32(0xFFFFF000)
    return r.view(np.float32)


def build_nc():
    import concourse.bass as bass
    import concourse.mybir as mybir
    import concourse.tile as tile
    from concourse import bacc

    F32 = mybir.dt.float32
    F32R = mybir.dt.float32r
    AF = mybir.ActivationFunctionType
    OP = mybir.AluOpType

    nc = bacc.Bacc(None, target_bir_lowering=False)

    # ---------- I/O ----------
    x_d = nc.dram_tensor("x_r", [C, N], F32R, kind="ExternalInput")
    xq_d = nc.dram_tensor("xq_r", [C, NQ], F32R, kind="ExternalInput")
    w_d = {}
    b_d = {}
    for nm in ("wq", "wk", "wv", "wp"):
        w_d[nm] = nc.dram_tensor(nm + "_t", [C, C], F32, kind="ExternalInput")
    for nm in ("bq", "bk", "bv", "bp"):
        b_d[nm] = nc.dram_tensor(nm + "_v", [C, 1], F32, kind="ExternalInput")
    gam_d = nc.dram_tensor("gamma_v", [C, 1], F32, kind="ExternalInput")
    bet_d = nc.dram_tensor("beta_v", [C, 1], F32, kind="ExternalInput")
    y_d = nc.dram_tensor("y", [C, NQ], F32, kind="ExternalOutput")

    # constants (fp32r via bitcast; all values exactly representable)
    ind1_np = np.zeros((P, 4), dtype=np.float32)
    for c in range(P):
        ind1_np[c, c // GSIZE] = 1.0 / GSIZE
    ind2_np = np.zeros((4, P), dtype=np.float32)
    for c in range(P):
        ind2_np[c // GSIZE, c] = 1.0
    ind1_d = nc.inline_tensor(ind1_np, name="ind1").bitcast(F32R)
    ind2_d = nc.inline_tensor(ind2_np, name="ind2").bitcast(F32R)
    allones_d = nc.inline_tensor(np.ones((P, P), np.float32), name="allones").bitcast(F32R)

    NI = N // P        # 32 key-token chunks

    with tile.TileContext(nc) as tc:
        with tc.tile_pool(name="persist", bufs=1) as pp, \
             tc.tile_pool(name="small", bufs=2) as sp, \
             tc.tile_pool(name="work", bufs=2) as wkp, \
             tc.tile_pool(name="etp", bufs=2) as etp, \
             tc.tile_pool(name="psA", bufs=2, space="PSUM") as psA, \
             tc.tile_pool(name="psB", bufs=2, space="PSUM") as psB:

            # ---------- load ----------
            xs = []
            xqs = []
            for cc in range(CC):
                t = pp.tile([P, N], F32R, name=f"xs{cc}")
                for j in range(8):  # chunked so GN stats can start early
                    nc.sync.dma_start(out=t[:, j * 512:(j + 1) * 512],
                                      in_=x_d[cc * P:(cc + 1) * P, j * 512:(j + 1) * 512])
                xs.append(t)
                tq = pp.tile([P, NQ], F32R, name=f"xqs{cc}")
                nc.sync.dma_start(out=tq, in_=xq_d[cc * P:(cc + 1) * P, :])
                xqs.append(tq)
            wraw = {}
            for nm in ("wq", "wk", "wv", "wp"):
                for cc in range(CC):
                    t = pp.tile([P, C], F32, name=f"{nm}raw{cc}")
                    nc.sync.dma_start(out=t, in_=w_d[nm][cc * P:(cc + 1) * P, :])
                    wraw[(nm, cc)] = t
            vecs = {}
            for nm, d in (("bq", b_d["bq"]), ("bk", b_d["bk"]), ("bv", b_d["bv"]),
                          ("bp", b_d["bp"]), ("gam", gam_d), ("bet", bet_d)):
                for cc in range(CC):
                    t = pp.tile([P, 1], F32, name=f"{nm}v{cc}")
                    nc.sync.dma_start(out=t, in_=d[cc * P:(cc + 1) * P, :])
                    vecs[(nm, cc)] = t
            ind1_s = pp.tile([P, 4], F32R, name="ind1s")
            nc.sync.dma_start(out=ind1_s, in_=ind1_d[:, :])
            ind2_s = pp.tile([4, P], F32R, name="ind2s")
            nc.sync.dma_start(out=ind2_s, in_=ind2_d[:, :])
            allones_s = pp.tile([P, P], F32R, name="allones_s")
            nc.sync.dma_start(out=allones_s, in_=allones_d[:, :])
            eps4 = pp.tile([4, 1], F32, name="eps4")
            nc.vector.memset(eps4, EPS)

            # ---------- GroupNorm stats -> per-channel scale/shift ----------
            s_vecs = []   # [128,1] f32 per cc: s_c = rstd_g * gamma_c
            t_vecs = []   # [128,1] f32 per cc: t_c = beta_c - mean_g * s_c
            for cc in range(CC):
                eng = nc.vector
                xf = xs[cc].bitcast(F32)
                stats = sp.tile([P, 8, 6], F32, name="bnstats")
                for sg in range(8):
                    eng.bn_stats(out=stats[:, sg, :], in_=xf[:, sg * 512:(sg + 1) * 512])
                mv = sp.tile([P, 2], F32, name="bnmv")
                eng.bn_aggr(out=mv, in_=stats)
                # st2 = (mean, E[x^2]) per channel, as fp32r
                m2 = sp.tile([P, 1], F32, name="gnm2")
                eng.tensor_mul(out=m2, in0=mv[:, 0:1], in1=mv[:, 0:1])
                st2 = sp.tile([P, 2], F32R, name="gnst2")
                eng.tensor_copy(out=st2[:, 0:1], in_=mv[:, 0:1])
                eng.tensor_tensor(out=st2[:, 1:2], in0=mv[:, 1:2], in1=m2, op=OP.add)
                # group means of (mean, E[x^2]) via indicator matmul
                pg = psB.tile([4, 2], F32, name="psg", tag="pav")
                nc.tensor.matmul(pg, ind1_s, st2, start=True, stop=True)
                pgs = sp.tile([4, 2], F32, name="gnpgs")
                eng.tensor_copy(out=pgs, in_=pg)
                gm2 = sp.tile([4, 1], F32, name="gngm2")
                eng.tensor_mul(out=gm2, in0=pgs[:, 0:1], in1=pgs[:, 0:1])
                gvar = sp.tile([4, 1], F32, name="gnvar")
                eng.tensor_tensor(out=gvar, in0=pgs[:, 1:2], in1=gm2, op=OP.subtract)
                gstd = sp.tile([4, 1], F32, name="gnstd")
                nc.scalar.activation(out=gstd, in_=gvar, func=AF.Sqrt, bias=eps4, scale=1.0)
                grstd = sp.tile([4, 1], F32, name="gnrstd")
                nc.vector.reciprocal(out=grstd, in_=gstd)
                gvals = sp.tile([4, 2], F32R, name="gnvals")
                eng.tensor_copy(out=gvals[:, 0:1], in_=pgs[:, 0:1])
                eng.tensor_copy(out=gvals[:, 1:2], in_=grstd)
                # broadcast group (mean, rstd) back to channels
                pb = psB.tile([P, 2], F32, name="psb2", tag="pav")
                nc.tensor.matmul(pb, ind2_s, gvals, start=True, stop=True)
                s_v = sp.tile([P, 1], F32, name="gns")
                eng.tensor_mul(out=s_v, in0=pb[:, 1:2], in1=vecs[("gam", cc)])
                ms = sp.tile([P, 1], F32, name="gnms")
                eng.tensor_mul(out=ms, in0=pb[:, 0:1], in1=s_v)
                t_v = sp.tile([P, 1], F32, name="gnt")
                eng.tensor_tensor(out=t_v, in0=vecs[("bet", cc)], in1=ms, op=OP.subtract)
                s_vecs.append(s_v)
                t_vecs.append(t_v)

            # ---------- fold GN into weights; effective biases ----------
            wr = {}
            for nm in ("wq", "wk", "wv"):
                for cc in range(CC):
                    t = pp.tile([P, C], F32R, name=f"{nm}r{cc}")
                    nc.vector.tensor_scalar_mul(out=t, in0=wraw[(nm, cc)], scalar1=s_vecs[cc])
                    wr[(nm, cc)] = t
            for cc in range(CC):
                t = pp.tile([P, C], F32R, name=f"wpr{cc}")
                nc.vector.tensor_copy(out=t, in_=wraw[("wp", cc)])
                wr[("wp", cc)] = t

            beff = {}
            for nm in ("wq", "wk", "wv"):
                bnm = "b" + nm[1]
                for oc in range(CC):
                    pbx = psB.tile([P, 1], F32, name="psbias", tag="pav")
                    for cc in range(CC):
                        # raw (unfolded) weights: bias is w @ t, not (w*s) @ t.
                        # fp32 matmul is fine here (N=1).
                        nc.tensor.matmul(pbx, wraw[(nm, cc)][:, oc * P:(oc + 1) * P],
                                         t_vecs[cc], start=(cc == 0), stop=(cc == CC - 1))
                    t = pp.tile([P, 1], F32, name=f"beff_{nm}{oc}")
                    nc.scalar.activation(out=t, in_=pbx, func=AF.Identity,
                                         bias=vecs[(bnm, oc)], scale=1.0)
                    beff[(nm, oc)] = t
            # b* = wp @ bv_eff + bp (V bias folded through the projection)
            for oc in range(CC):
                pbx = psB.tile([P, 1], F32, name="psbias2", tag="pav")
                for cc in range(CC):
                    nc.tensor.matmul(pbx, wraw[("wp", cc)][:, oc * P:(oc + 1) * P],
                                     beff[("wv", cc)], start=(cc == 0), stop=(cc == CC - 1))
                t = pp.tile([P, 1], F32, name=f"bstar{oc}")
                nc.scalar.activation(out=t, in_=pbx, func=AF.Identity,
                                     bias=vecs[("bp", oc)], scale=1.0)
                beff[("wp", oc)] = t

            # ---------- projections ----------
            Qs = [pp.tile([P, NQ], F32R, name=f"Q{oc}") for oc in range(CC)]
            Ks = [pp.tile([P, N], F32R, name=f"K{oc}") for oc in range(CC)]
            for oc in range(CC):
                for i in range(NQ // QW):
                    pq = psA.tile([P, QW], F32, name="psq", tag="pst")
                    for h in range(2):
                        sl = slice(i * QW + h * 512, i * QW + (h + 1) * 512)
                        for cc in range(CC):
                            nc.tensor.matmul(pq[:, h * 512:(h + 1) * 512],
                                             wr[("wq", cc)][:, oc * P:(oc + 1) * P],
                                             xqs[cc][:, sl],
                                             start=(cc == 0), stop=(cc == CC - 1))
                    nc.scalar.activation(out=Qs[oc][:, i * QW:(i + 1) * QW], in_=pq,
                                         func=AF.Identity, bias=beff[("wq", oc)], scale=1.0)
                for i in range(N // QW):
                    pk = psA.tile([P, QW], F32, name="psk", tag="pst")
                    for h in range(2):
                        sl = slice(i * QW + h * 512, i * QW + (h + 1) * 512)
                        for cc in range(CC):
                            nc.tensor.matmul(pk[:, h * 512:(h + 1) * 512],
                                             wr[("wk", cc)][:, oc * P:(oc + 1) * P],
                                             xs[cc][:, sl],
                                             start=(cc == 0), stop=(cc == CC - 1))
                    nc.scalar.activation(out=Ks[oc][:, i * QW:(i + 1) * QW], in_=pk,
                                         func=AF.Identity, bias=beff[("wk", oc)], scale=1.0)
            VTs = pp.tile([P, NI * C], F32R, name="VTs")  # [128 tok, 32*256]
            for it in range(0, NI, 2):
                pv = psA.tile([P, 512], F32, name="psv", tag="pst")
                for j in range(2):
                    for cc in range(CC):
                        nc.tensor.matmul(pv[:, j * C:(j + 1) * C],
                                         xs[cc][:, (it + j) * P:(it + j + 1) * P],
                                         wr[("wv", cc)],
                                         start=(cc == 0), stop=(cc == CC - 1))
                nc.vector.tensor_copy(out=VTs[:, it * C:(it + 2) * C], in_=pv)

            # ---------- attention (QW=1024 query columns per pass) ----------
            for qp in range(NQP):
                pav = [psB.tile([P, QW], F32, name=f"pav{cc}", tag="pav") for cc in range(CC)]
                acc = etp.tile([P, QW], F32R, name="acc", tag="acc")
                accf = acc.bitcast(F32)
                for m in range(NI):
                    pst = psA.tile([P, QW], F32, name="pst", tag="pst")
                    for h in range(2):
                        for oc in range(CC):
                            nc.tensor.matmul(pst[:, h * 512:(h + 1) * 512],
                                             Ks[oc][:, m * P:(m + 1) * P],
                                             Qs[oc][:, qp * QW + h * 512:qp * QW + (h + 1) * 512],
                                             start=(oc == 0), stop=(oc == CC - 1))
                    et = etp.tile([P, QW], F32R, name="et", tag="et")
                    nc.scalar.activation(out=et, in_=pst, func=AF.Exp)
                    for h in range(2):
                        for cc in range(CC):
                            nc.tensor.matmul(pav[cc][:, h * 512:(h + 1) * 512],
                                             VTs[:, m * C + cc * P: m * C + (cc + 1) * P],
                                             et[:, h * 512:(h + 1) * 512],
                                             start=(m == 0), stop=(m == NI - 1))
                    if m == 0:
                        nc.vector.tensor_copy(out=acc, in_=et)
                    else:
                        nc.vector.tensor_tensor(out=acc, in0=accf, in1=et.bitcast(F32),
                                                op=OP.add)
                # denominator -> broadcast reciprocal
                pd = psA.tile([P, QW], F32, name="psd", tag="pst")
                for h in range(2):
                    nc.tensor.matmul(pd[:, h * 512:(h + 1) * 512], allones_s,
                                     acc[:, h * 512:(h + 1) * 512], start=True, stop=True)
                rb = wkp.tile([P, QW], F32, name="rb", tag="rb")
                nc.vector.reciprocal(out=rb, in_=pd)
                obar = []
                for cc in range(CC):
                    ob = wkp.tile([P, QW], F32R, name="obar", tag="obar")
                    nc.vector.tensor_tensor(out=ob, in0=pav[cc], in1=rb, op=OP.mult)
                    obar.append(ob)
                for oc in range(CC):
                    py = psA.tile([P, QW], F32, name="psy", tag="pst")
                    for h in range(2):
                        for cc in range(CC):
                            nc.tensor.matmul(py[:, h * 512:(h + 1) * 512],
                                             wr[("wp", cc)][:, oc * P:(oc + 1) * P],
                                             obar[cc][:, h * 512:(h + 1) * 512],
                                             start=(cc == 0), stop=(cc == CC - 1))
                    y2 = wkp.tile([P, QW], F32, name="y2", tag="y2")
                    nc.vector.scalar_tensor_tensor(
                        out=y2, in0=py, scalar=beff[("wp", oc)],
                        in1=xqs[oc].bitcast(F32)[:, qp * QW:(qp + 1) * QW],
                        op0=OP.add, op1=OP.add)
                    nc.sync.dma_start(out=y_d[oc * P:(oc + 1) * P, qp * QW:(qp + 1) * QW],
                                      in_=y2)

    nc.finalize()
    return nc


def _get_nc():
    if "nc" not in _cache:
        _cache["nc"] = build_nc()
    return _cache["nc"]


def make_in_maps(x, gamma, beta, wq, bq, wk, bk, wv, bv, wp, bp):
    x = np.ascontiguousarray(np.asarray(x, dtype=np.float32))
    f32 = lambda a: np.ascontiguousarray(np.asarray(a, dtype=np.float32))
    scale = 1.0 / np.sqrt(np.float32(C))
    shared = {
        "wq_t": f32(np.asarray(wq, np.float32).T),
        "wk_t": f32(np.asarray(wk, np.float32).T * scale),
        "wv_t": f32(np.asarray(wv, np.float32).T),
        "wp_t": f32(np.asarray(wp, np.float32).T),
        "bq_v": f32(bq).reshape(C, 1),
        "bk_v": f32(np.asarray(bk, np.float32) * scale).reshape(C, 1),
        "bv_v": f32(bv).reshape(C, 1),
        "bp_v": f32(bp).reshape(C, 1),
        "gamma_v": f32(gamma).reshape(C, 1),
        "beta_v": f32(beta).reshape(C, 1),
    }
    in_maps = []
    for core in range(NCORES):
        bi, half = core // 2, core % 2
        x_r = round_tf32(x[bi].reshape(C, N))
        xq_r = np.ascontiguousarray(x_r[:, half * NQ:(half + 1) * NQ])
        m = dict(shared)
        m["x_r"] = x_r
        m["xq_r"] = xq_r
        in_maps.append(m)
    return in_maps


def run(inputs: dict, trace: bool = False):
    from concourse.bass_utils import run_bass_kernel_spmd
    nc = _get_nc()
    in_maps = make_in_maps(**inputs)
    res = run_bass_kernel_spmd(nc, in_maps, core_ids=list(range(NCORES)), trace=trace)
    y = np.empty((B, C, N), dtype=np.float32)
    for core in range(NCORES):
        bi, half = core // 2, core % 2
        y[bi][:, half * NQ:(half + 1) * NQ] = res.results[core]["y"]
    return y.reshape(B, C, H, W), res


def kernel(**inputs) -> np.ndarray:
    out, _ = run(inputs, trace=False)
    return out


# revision 10
# speedup vs baseline: 1.2231x; 1.2231x over previous
"""AttentionBlock (GroupNorm + single-head self-attention + proj + residual)
on 8 Trainium2 NeuronCores.

Sharding: batch (4) x query-token-half (2) -> 8 shards. Each core gets the
full image of its batch element (for GroupNorm stats and K/V over all 4096
tokens) plus its half of the query tokens; K/V/GN are computed redundantly
by the 2 cores sharing a batch element, which is far cheaper than
cross-core collectives at this size.

Math per core (c=256 channels, n=4096 tokens, nq=2048 query tokens):
  GroupNorm is folded into the QKV weights: xn = s_c * x + t_c with
  per-channel s,t computed on-device from group stats, so
  Q = (wq*s) @ x + (wq@t + bq), etc. The score scale 1/sqrt(c) is folded
  into wk/bk on the host. The V-path bias is folded into the output
  projection bias (b* = wp@(wv@t+bv) + bp).
  Scores are computed k-major: S^T[m,i] = sum_o K[o,m] Q[o,i] so softmax
  needs a cross-partition denominator, obtained by accumulating exp tiles
  on DVE and one all-ones matmul (which also broadcasts the sums to all
  partitions); A@V uses lhsT = V^T (computed directly as x^T @ wv') so no
  transposes are needed anywhere. The attention loop processes all 1024
  query columns of a half-shard at once (2-bank PSUM tiles) to halve the
  ACT/DVE instruction count.

All matmuls run in float32r (TF32-like: fp32 with 11-bit mantissa, full
fp32 accumulate) which streams near bf16 rate -- measured ~2e-4 relative
error on the full block, ~15x better than bf16.
"""

import numpy as np

B, C, H, W = 4, 256, 64, 64
N = H * W            # 4096 tokens
NQ = N // 2          # 2048 query tokens per core
GROUPS = 8
GSIZE = C // GROUPS  # 32 channels per group
EPS = 1e-5
P = 128              # partitions
CC = C // P          # 2 channel chunks
NCORES = 8
QW = 1024            # query columns processed per attention pass
NQP = NQ // QW       # 2 passes

_cache = {}


def round_tf32(x: np.ndarray) -> np.ndarray:
    """Round fp32 to fp32r (11-bit mantissa, round-to-nearest-even)."""
    i = np.ascontiguousarray(x, dtype=np.float32).view(np.uint32)
    r = (i + np.uint32(0x7FF) + ((i >> np.uint32(12)) & np.uint32(1))) & np.uint32(0xFFFFF000)
    return r.view(np.float32)


def build_nc():
    import concourse.bass as bass
    import concourse.mybir as mybir
    import concourse.tile as tile
    from concourse import bacc

    F32 = mybir.dt.float32
    F32R = mybir.dt.float32r
    AF = mybir.ActivationFunctionType
    OP = mybir.AluOpType

    nc = bacc.Bacc(None, target_bir_lowering=False)

    # ---------- I/O ----------
    x_d = nc.dram_tensor("x_r", [C, N], F32R, kind="ExternalInput")
    xq_d = nc.dram_tensor("xq_r", [C, NQ], F32R, kind="ExternalInput")
    w_d = {}
    b_d = {}
    for nm in ("wq", "wk", "wv", "wp"):
        w_d[nm] = nc.dram_tensor(nm + "_t", [C, C], F32, kind="ExternalInput")
    for nm in ("bq", "bk", "bv", "bp"):
        b_d[nm] = nc.dram_tensor(nm + "_v", [C, 1], F32, kind="ExternalInput")
    gam_d = nc.dram_tensor("gamma_v", [C, 1], F32, kind="ExternalInput")
    bet_d = nc.dram_tensor("beta_v", [C, 1], F32, kind="ExternalInput")
    y_d = nc.dram_tensor("y", [C, NQ], F32, kind="ExternalOutput")

    # constants (fp32r via bitcast; all values exactly representable)
    ind1_np = np.zeros((P, 4), dtype=np.float32)
    for c in range(P):
        ind1_np[c, c // GSIZE] = 1.0 / GSIZE
    ind2_np = np.zeros((4, P), dtype=np.float32)
    for c in range(P):
        ind2_np[c // GSIZE, c] = 1.0
    ind1_d = nc.inline_tensor(ind1_np, name="ind1").bitcast(F32R)
    ind2_d = nc.inline_tensor(ind2_np, name="ind2").bitcast(F32R)
    allones_d = nc.inline_tensor(np.ones((P, P), np.float32), name="allones").bitcast(F32R)

    NI = N // P        # 32 key-token chunks

    with tile.TileContext(nc) as tc:
        with tc.tile_pool(name="persist", bufs=1) as pp, \
             tc.tile_pool(name="small", bufs=2) as sp, \
             tc.tile_pool(name="work", bufs=2) as wkp, \
             tc.tile_pool(name="etp", bufs=2) as etp, \
             tc.tile_pool(name="psA", bufs=2, space="PSUM") as psA, \
             tc.tile_pool(name="psB", bufs=2, space="PSUM") as psB:

            # ---------- load ----------
            xs = []
            xqs = []
            for cc in range(CC):
                t = pp.tile([P, N], F32R, name=f"xs{cc}")
                for j in range(8):  # chunked so GN stats can start early
                    nc.sync.dma_start(out=t[:, j * 512:(j + 1) * 512],
                                      in_=x_d[cc * P:(cc + 1) * P, j * 512:(j + 1) * 512])
                xs.append(t)
            wraw = {}
            for nm in ("wq", "wk", "wv", "wp"):
                for cc in range(CC):
                    t = pp.tile([P, C], F32, name=f"{nm}raw{cc}")
                    nc.sync.dma_start(out=t, in_=w_d[nm][cc * P:(cc + 1) * P, :])
                    wraw[(nm, cc)] = t
            vecs = {}
            for nm, d in (("bq", b_d["bq"]), ("bk", b_d["bk"]), ("bv", b_d["bv"]),
                          ("bp", b_d["bp"]), ("gam", gam_d), ("bet", bet_d)):
                for cc in range(CC):
                    t = pp.tile([P, 1], F32, name=f"{nm}v{cc}")
                    nc.sync.dma_start(out=t, in_=d[cc * P:(cc + 1) * P, :])
                    vecs[(nm, cc)] = t
            ind1_s = pp.tile([P, 4], F32R, name="ind1s")
            nc.sync.dma_start(out=ind1_s, in_=ind1_d[:, :])
            ind2_s = pp.tile([4, P], F32R, name="ind2s")
            nc.sync.dma_start(out=ind2_s, in_=ind2_d[:, :])
            allones_s = pp.tile([P, P], F32R, name="allones_s")
            nc.sync.dma_start(out=allones_s, in_=allones_d[:, :])
            eps4 = pp.tile([4, 1], F32, name="eps4")
            nc.vector.memset(eps4, EPS)
            for cc in range(CC):
                tq = pp.tile([P, NQ], F32R, name=f"xqs{cc}")
                nc.sync.dma_start(out=tq, in_=xq_d[cc * P:(cc + 1) * P, :])
                xqs.append(tq)

            # ---------- GroupNorm stats -> per-channel scale/shift ----------
            s_vecs = []   # [128,1] f32 per cc: s_c = rstd_g * gamma_c
            t_vecs = []   # [128,1] f32 per cc: t_c = beta_c - mean_g * s_c
            for cc in range(CC):
                eng = nc.vector
                xf = xs[cc].bitcast(F32)
                stats = sp.tile([P, 8, 6], F32, name="bnstats")
                for sg in range(8):
                    eng.bn_stats(out=stats[:, sg, :], in_=xf[:, sg * 512:(sg + 1) * 512])
                mv = sp.tile([P, 2], F32, name="bnmv")
                eng.bn_aggr(out=mv, in_=stats)
                # st2 = (mean, E[x^2]) per channel, as fp32r
                m2 = sp.tile([P, 1], F32, name="gnm2")
                eng.tensor_mul(out=m2, in0=mv[:, 0:1], in1=mv[:, 0:1])
                st2 = sp.tile([P, 2], F32R, name="gnst2")
                eng.tensor_copy(out=st2[:, 0:1], in_=mv[:, 0:1])
                eng.tensor_tensor(out=st2[:, 1:2], in0=mv[:, 1:2], in1=m2, op=OP.add)
                # group means of (mean, E[x^2]) via indicator matmul
                pg = psB.tile([4, 2], F32, name="psg", tag="pav")
                nc.tensor.matmul(pg, ind1_s, st2, start=True, stop=True)
                pgs = sp.tile([4, 2], F32, name="gnpgs")
                eng.tensor_copy(out=pgs, in_=pg)
                gm2 = sp.tile([4, 1], F32, name="gngm2")
                eng.tensor_mul(out=gm2, in0=pgs[:, 0:1], in1=pgs[:, 0:1])
                gvar = sp.tile([4, 1], F32, name="gnvar")
                eng.tensor_tensor(out=gvar, in0=pgs[:, 1:2], in1=gm2, op=OP.subtract)
                gstd = sp.tile([4, 1], F32, name="gnstd")
                nc.scalar.activation(out=gstd, in_=gvar, func=AF.Sqrt, bias=eps4, scale=1.0)
                grstd = sp.tile([4, 1], F32, name="gnrstd")
                nc.vector.reciprocal(out=grstd, in_=gstd)
                gvals = sp.tile([4, 2], F32R, name="gnvals")
                eng.tensor_copy(out=gvals[:, 0:1], in_=pgs[:, 0:1])
                eng.tensor_copy(out=gvals[:, 1:2], in_=grstd)
                # broadcast group (mean, rstd) back to channels
                pb = psB.tile([P, 2], F32, name="psb2", tag="pav")
                nc.tensor.matmul(pb, ind2_s, gvals, start=True, stop=True)
                s_v = sp.tile([P, 1], F32, name="gns")
                eng.tensor_mul(out=s_v, in0=pb[:, 1:2], in1=vecs[("gam", cc)])
                ms = sp.tile([P, 1], F32, name="gnms")
                eng.tensor_mul(out=ms, in0=pb[:, 0:1], in1=s_v)
                t_v = sp.tile([P, 1], F32, name="gnt")
                eng.tensor_tensor(out=t_v, in0=vecs[("bet", cc)], in1=ms, op=OP.subtract)
                s_vecs.append(s_v)
                t_vecs.append(t_v)

            # ---------- fold GN into weights; effective biases ----------
            wr = {}
            for nm in ("wq", "wk", "wv"):
                for cc in range(CC):
                    t = pp.tile([P, C], F32R, name=f"{nm}r{cc}")
                    nc.vector.tensor_scalar_mul(out=t, in0=wraw[(nm, cc)], scalar1=s_vecs[cc])
                    wr[(nm, cc)] = t
            for cc in range(CC):
                t = pp.tile([P, C], F32R, name=f"wpr{cc}")
                nc.vector.tensor_copy(out=t, in_=wraw[("wp", cc)])
                wr[("wp", cc)] = t

            beff = {}
            for nm in ("wq", "wk", "wv"):
                bnm = "b" + nm[1]
                for oc in range(CC):
                    pbx = psB.tile([P, 1], F32, name="psbias", tag="pav")
                    for cc in range(CC):
                        # raw (unfolded) weights: bias is w @ t, not (w*s) @ t.
                        # fp32 matmul is fine here (N=1).
                        nc.tensor.matmul(pbx, wraw[(nm, cc)][:, oc * P:(oc + 1) * P],
                                         t_vecs[cc], start=(cc == 0), stop=(cc == CC - 1))
                    t = pp.tile([P, 1], F32, name=f"beff_{nm}{oc}")
                    nc.scalar.activation(out=t, in_=pbx, func=AF.Identity,
                                         bias=vecs[(bnm, oc)], scale=1.0)
                    beff[(nm, oc)] = t
            # b* = wp @ bv_eff + bp (V bias folded through the projection)
            for oc in range(CC):
                pbx = psB.tile([P, 1], F32, name="psbias2", tag="pav")
                for cc in range(CC):
                    nc.tensor.matmul(pbx, wraw[("wp", cc)][:, oc * P:(oc + 1) * P],
                                     beff[("wv", cc)], start=(cc == 0), stop=(cc == CC - 1))
                t = pp.tile([P, 1], F32, name=f"bstar{oc}")
                nc.scalar.activation(out=t, in_=pbx, func=AF.Identity,
                                     bias=vecs[("bp", oc)], scale=1.0)
                beff[("wp", oc)] = t

            # ---------- projections ----------
            Qs = [pp.tile([P, NQ], F32R, name=f"Q{oc}") for oc in range(CC)]
            Ks = [pp.tile([P, N], F32R, name=f"K{oc}") for oc in range(CC)]
            for oc in range(CC):
                for i in range(NQ // QW):
                    pq = psA.tile([P, QW], F32, name="psq", tag="pst")
                    for h in range(2):
                        sl = slice(i * QW + h * 512, i * QW + (h + 1) * 512)
                        for cc in range(CC):
                            nc.tensor.matmul(pq[:, h * 512:(h + 1) * 512],
                                             wr[("wq", cc)][:, oc * P:(oc + 1) * P],
                                             xqs[cc][:, sl],
                                             start=(cc == 0), stop=(cc == CC - 1))
                    nc.scalar.activation(out=Qs[oc][:, i * QW:(i + 1) * QW], in_=pq,
                                         func=AF.Identity, bias=beff[("wq", oc)], scale=1.0)
                for i in range(N // QW):
                    pk = psA.tile([P, QW], F32, name="psk", tag="pst")
                    for h in range(2):
                        sl = slice(i * QW + h * 512, i * QW + (h + 1) * 512)
                        for cc in range(CC):
                            nc.tensor.matmul(pk[:, h * 512:(h + 1) * 512],
                                             wr[("wk", cc)][:, oc * P:(oc + 1) * P],
                                             xs[cc][:, sl],
                                             start=(cc == 0), stop=(cc == CC - 1))
                    nc.scalar.activation(out=Ks[oc][:, i * QW:(i + 1) * QW], in_=pk,
                                         func=AF.Identity, bias=beff[("wk", oc)], scale=1.0)
            VTs = pp.tile([P, NI * C], F32R, name="VTs")  # [128 tok, 32*256]
            for it in range(0, NI, 2):
                pv = psA.tile([P, 512], F32, name="psv", tag="pst")
                for j in range(2):
                    for cc in range(CC):
                        nc.tensor.matmul(pv[:, j * C:(j + 1) * C],
                                         xs[cc][:, (it + j) * P:(it + j + 1) * P],
                                         wr[("wv", cc)],
                                         start=(cc == 0), stop=(cc == CC - 1))
                nc.vector.tensor_copy(out=VTs[:, it * C:(it + 2) * C], in_=pv)

            # ---------- attention (QW=1024 query columns per pass) ----------
            for qp in range(NQP):
                pav = [psB.tile([P, QW], F32, name=f"pav{cc}", tag="pav") for cc in range(CC)]
                acc = etp.tile([P, QW], F32R, name="acc", tag="acc")
                accf = acc.bitcast(F32)
                acc_engs = [nc.vector, nc.gpsimd]
                for m in range(NI):
                    pst = psA.tile([P, QW], F32, name="pst", tag="pst")
                    for h in range(2):
                        for oc in range(CC):
                            nc.tensor.matmul(pst[:, h * 512:(h + 1) * 512],
                                             Ks[oc][:, m * P:(m + 1) * P],
                                             Qs[oc][:, qp * QW + h * 512:qp * QW + (h + 1) * 512],
                                             start=(oc == 0), stop=(oc == CC - 1))
                    et = etp.tile([P, QW], F32R, name="et", tag="et")
                    nc.scalar.activation(out=et, in_=pst, func=AF.Exp)
                    for h in range(2):
                        for cc in range(CC):
                            nc.tensor.matmul(pav[cc][:, h * 512:(h + 1) * 512],
                                             VTs[:, m * C + cc * P: m * C + (cc + 1) * P],
                                             et[:, h * 512:(h + 1) * 512],
                                             start=(m == 0), stop=(m == NI - 1))
                    for h in range(2):
                        sl = slice(h * 512, (h + 1) * 512)
                        if m == 0:
                            acc_engs[h].tensor_copy(out=acc[:, sl], in_=et[:, sl])
                        else:
                            acc_engs[h].tensor_tensor(out=acc[:, sl], in0=accf[:, sl],
                                                      in1=et.bitcast(F32)[:, sl], op=OP.add)
                # denominator -> broadcast reciprocal
                pd = psA.tile([P, QW], F32, name="psd", tag="pst")
                for h in range(2):
                    nc.tensor.matmul(pd[:, h * 512:(h + 1) * 512], allones_s,
                                     acc[:, h * 512:(h + 1) * 512], start=True, stop=True)
                rb = wkp.tile([P, QW], F32, name="rb", tag="rb")
                nc.vector.reciprocal_approx_fast(out=rb, in_=pd)
                obar = []
                for cc in range(CC):
                    ob = wkp.tile([P, QW], F32R, name="obar", tag="obar")
                    nc.vector.tensor_tensor(out=ob, in0=pav[cc], in1=rb, op=OP.mult)
                    obar.append(ob)
                for oc in range(CC):
                    py = psA.tile([P, QW], F32, name="psy", tag="pst")
                    for h in range(2):
                        for cc in range(CC):
                            nc.tensor.matmul(py[:, h * 512:(h + 1) * 512],
                                             wr[("wp", cc)][:, oc * P:(oc + 1) * P],
                                             obar[cc][:, h * 512:(h + 1) * 512],
                                             start=(cc == 0), stop=(cc == CC - 1))
                    y2 = wkp.tile([P, QW], F32, name="y2", tag="y2")
                    nc.vector.scalar_tensor_tensor(
                        out=y2, in0=py, scalar=beff[("wp", oc)],
                        in1=xqs[oc].bitcast(F32)[:, qp * QW:(qp + 1) * QW],
                        op0=OP.add, op1=OP.add)
                    nc.sync.dma_start(out=y_d[oc * P:(oc + 1) * P, qp * QW:(qp + 1) * QW],
                                      in_=y2)

    nc.finalize()
    return nc


def _get_nc():
    if "nc" not in _cache:
        _cache["nc"] = build_nc()
    return _cache["nc"]


def make_in_maps(x, gamma, beta, wq, bq, wk, bk, wv, bv, wp, bp):
    x = np.ascontiguousarray(np.asarray(x, dtype=np.float32))
    f32 = lambda a: np.ascontiguousarray(np.asarray(a, dtype=np.float32))
    scale = 1.0 / np.sqrt(np.float32(C))
    shared = {
        "wq_t": f32(np.asarray(wq, np.float32).T),
        "wk_t": f32(np.asarray(wk, np.float32).T * scale),
        "wv_t": f32(np.asarray(wv, np.float32).T),
        "wp_t": f32(np.asarray(wp, np.float32).T),
        "bq_v": f32(bq).reshape(C, 1),
        "bk_v": f32(np.asarray(bk, np.float32) * scale).reshape(C, 1),
        "bv_v": f32(bv).reshape(C, 1),
        "bp_v": f32(bp).reshape(C, 1),
        "gamma_v": f32(gamma).reshape(C, 1),
        "beta_v": f32(beta).reshape(C, 1),
    }
    in_maps = []
    for core in range(NCORES):
        bi, half = core // 2, core % 2
        x_r = round_tf32(x[bi].reshape(C, N))
        xq_r = np.ascontiguousarray(x_r[:, half * NQ:(half + 1) * NQ])
        m = dict(shared)
        m["x_r"] = x_r
        m["xq_r"] = xq_r
        in_maps.append(m)
    return in_maps


def run(inputs: dict, trace: bool = False):
    from concourse.bass_utils import run_bass_kernel_spmd
    nc = _get_nc()
    in_maps = make_in_maps(**inputs)
    res = run_bass_kernel_spmd(nc, in_maps, core_ids=list(range(NCORES)), trace=trace)
    y = np.empty((B, C, N), dtype=np.float32)
    for core in range(NCORES):
        bi, half = core // 2, core % 2
        y[bi][:, half * NQ:(half + 1) * NQ] = res.results[core]["y"]
    return y.reshape(B, C, H, W), res


def kernel(**inputs) -> np.ndarray:
    out, _ = run(inputs, trace=False)
    return out


# revision 11
# speedup vs baseline: 1.2263x; 1.0026x over previous
"""AttentionBlock (GroupNorm + single-head self-attention + proj + residual)
on 8 Trainium2 NeuronCores.

Sharding: batch (4) x query-token-half (2) -> 8 shards. Each core gets the
full image of its batch element (for GroupNorm stats and K/V over all 4096
tokens) plus its half of the query tokens; K/V/GN are computed redundantly
by the 2 cores sharing a batch element, which is far cheaper than
cross-core collectives at this size.

Math per core (c=256 channels, n=4096 tokens, nq=2048 query tokens):
  GroupNorm is folded into the QKV weights: xn = s_c * x + t_c with
  per-channel s,t computed on-device from group stats, so
  Q = (wq*s) @ x + (wq@t + bq), etc. The score scale 1/sqrt(c) is folded
  into wk/bk on the host. The V-path bias is folded into the output
  projection bias (b* = wp@(wv@t+bv) + bp).
  Scores are computed k-major: S^T[m,i] = sum_o K[o,m] Q[o,i] so softmax
  needs a cross-partition denominator, obtained by accumulating exp tiles
  on DVE and one all-ones matmul (which also broadcasts the sums to all
  partitions); A@V uses lhsT = V^T (computed directly as x^T @ wv') so no
  transposes are needed anywhere. The attention loop processes all 1024
  query columns of a half-shard at once (2-bank PSUM tiles) to halve the
  ACT/DVE instruction count.

All matmuls run in float32r (TF32-like: fp32 with 11-bit mantissa, full
fp32 accumulate) which streams near bf16 rate -- measured ~2e-4 relative
error on the full block, ~15x better than bf16.
"""

import numpy as np

B, C, H, W = 4, 256, 64, 64
N = H * W            # 4096 tokens
NQ = N // 2          # 2048 query tokens per core
GROUPS = 8
GSIZE = C // GROUPS  # 32 channels per group
EPS = 1e-5
P = 128              # partitions
CC = C // P          # 2 channel chunks
NCORES = 8
QW = 1024            # query columns processed per attention pass
NQP = NQ // QW       # 2 passes

_cache = {}


def round_tf32(x: np.ndarray) -> np.ndarray:
    """Round fp32 to fp32r (11-bit mantissa, round-to-nearest-even)."""
    i = np.ascontiguousarray(x, dtype=np.float32).view(np.uint32)
    r = (i + np.uint32(0x7FF) + ((i >> np.uint32(12)) & np.uint32(1))) & np.uint32(0xFFFFF000)
    return r.view(np.float32)


def build_nc():
    import concourse.bass as bass
    import concourse.mybir as mybir
    import concourse.tile as tile
    from concourse import bacc

    F32 = mybir.dt.float32
    F32R = mybir.dt.float32r
    AF = mybir.ActivationFunctionType
    OP = mybir.AluOpType

    nc = bacc.Bacc(None, target_bir_lowering=False)

    # ---------- I/O ----------
    x_d = nc.dram_tensor("x_r", [C, N], F32R, kind="ExternalInput")
    xq_d = nc.dram_tensor("xq_r", [C, NQ], F32R, kind="ExternalInput")
    w_d = {}
    b_d = {}
    for nm in ("wq", "wk", "wv", "wp"):
        w_d[nm] = nc.dram_tensor(nm + "_t", [C, C], F32, kind="ExternalInput")
    for nm in ("bq", "bk", "bv", "bp"):
        b_d[nm] = nc.dram_tensor(nm + "_v", [C, 1], F32, kind="ExternalInput")
    gam_d = nc.dram_tensor("gamma_v", [C, 1], F32, kind="ExternalInput")
    bet_d = nc.dram_tensor("beta_v", [C, 1], F32, kind="ExternalInput")
    y_d = nc.dram_tensor("y", [C, NQ], F32, kind="ExternalOutput")

    # constants (fp32r via bitcast; all values exactly representable)
    ind1_np = np.zeros((P, 4), dtype=np.float32)
    for c in range(P):
        ind1_np[c, c // GSIZE] = 1.0 / GSIZE
    ind2_np = np.zeros((4, P), dtype=np.float32)
    for c in range(P):
        ind2_np[c // GSIZE, c] = 1.0
    ind1_d = nc.inline_tensor(ind1_np, name="ind1").bitcast(F32R)
    ind2_d = nc.inline_tensor(ind2_np, name="ind2").bitcast(F32R)
    allones_d = nc.inline_tensor(np.ones((P, P), np.float32), name="allones").bitcast(F32R)

    NI = N // P        # 32 key-token chunks

    with tile.TileContext(nc) as tc:
        with tc.tile_pool(name="persist", bufs=1) as pp, \
             tc.tile_pool(name="small", bufs=2) as sp, \
             tc.tile_pool(name="work", bufs=2) as wkp, \
             tc.tile_pool(name="etp", bufs=2) as etp, \
             tc.tile_pool(name="psA", bufs=2, space="PSUM") as psA, \
             tc.tile_pool(name="psB", bufs=2, space="PSUM") as psB:

            # ---------- load ----------
            xs = []
            xqs = []
            for cc in range(CC):
                t = pp.tile([P, N], F32R, name=f"xs{cc}")
                for j in range(8):  # chunked so GN stats can start early
                    nc.sync.dma_start(out=t[:, j * 512:(j + 1) * 512],
                                      in_=x_d[cc * P:(cc + 1) * P, j * 512:(j + 1) * 512])
                xs.append(t)
            wraw = {}
            for nm in ("wq", "wk", "wv", "wp"):
                for cc in range(CC):
                    t = pp.tile([P, C], F32, name=f"{nm}raw{cc}")
                    nc.sync.dma_start(out=t, in_=w_d[nm][cc * P:(cc + 1) * P, :])
                    wraw[(nm, cc)] = t
            vecs = {}
            for nm, d in (("bq", b_d["bq"]), ("bk", b_d["bk"]), ("bv", b_d["bv"]),
                          ("bp", b_d["bp"]), ("gam", gam_d), ("bet", bet_d)):
                for cc in range(CC):
                    t = pp.tile([P, 1], F32, name=f"{nm}v{cc}")
                    nc.sync.dma_start(out=t, in_=d[cc * P:(cc + 1) * P, :])
                    vecs[(nm, cc)] = t
            ind1_s = pp.tile([P, 4], F32R, name="ind1s")
            nc.sync.dma_start(out=ind1_s, in_=ind1_d[:, :])
            ind2_s = pp.tile([4, P], F32R, name="ind2s")
            nc.sync.dma_start(out=ind2_s, in_=ind2_d[:, :])
            allones_s = pp.tile([P, P], F32R, name="allones_s")
            nc.sync.dma_start(out=allones_s, in_=allones_d[:, :])
            eps4 = pp.tile([4, 1], F32, name="eps4")
            nc.vector.memset(eps4, EPS)
            for cc in range(CC):
                tq = pp.tile([P, NQ], F32R, name=f"xqs{cc}")
                nc.sync.dma_start(out=tq, in_=xq_d[cc * P:(cc + 1) * P, :])
                xqs.append(tq)

            # ---------- GroupNorm stats -> per-channel scale/shift ----------
            # cc0 via DVE bn_stats; cc1 via ACT accumulate (sum, sum of
            # squares) so the two chains run on different engines at startup.
            s_vecs = []   # [128,1] f32 per cc: s_c = rstd_g * gamma_c
            t_vecs = []   # [128,1] f32 per cc: t_c = beta_c - mean_g * s_c
            for cc in range(CC):
                eng = nc.vector
                xf = xs[cc].bitcast(F32)
                st2 = sp.tile([P, 2], F32R, name="gnst2")
                if cc == 0:
                    stats = sp.tile([P, 8, 6], F32, name="bnstats")
                    for sg in range(8):
                        eng.bn_stats(out=stats[:, sg, :], in_=xf[:, sg * 512:(sg + 1) * 512])
                    mv = sp.tile([P, 2], F32, name="bnmv")
                    eng.bn_aggr(out=mv, in_=stats)
                    # st2 = (mean, E[x^2]) per channel, as fp32r
                    m2 = sp.tile([P, 1], F32, name="gnm2")
                    eng.tensor_mul(out=m2, in0=mv[:, 0:1], in1=mv[:, 0:1])
                    eng.tensor_copy(out=st2[:, 0:1], in_=mv[:, 0:1])
                    eng.tensor_tensor(out=st2[:, 1:2], in0=mv[:, 1:2], in1=m2, op=OP.add)
                else:
                    parts = sp.tile([P, 2, 8], F32, name="actparts")
                    scr = sp.tile([P, 512], F32, name="actscr")
                    for sg in range(8):
                        ch = xf[:, sg * 512:(sg + 1) * 512]
                        nc.scalar.activation(out=scr, in_=ch, func=AF.Identity,
                                             accum_out=parts[:, 0, sg:sg + 1])
                        scr2 = sp.tile([P, 512], F32, name="actscr2")
                        nc.scalar.activation(out=scr2, in_=ch, func=AF.Square,
                                             accum_out=parts[:, 1, sg:sg + 1])
                    sums = sp.tile([P, 2], F32, name="actsums")
                    eng.tensor_reduce(out=sums, in_=parts, op=OP.add,
                                      axis=mybir.AxisListType.X)
                    # mean = sum/n; E[x^2] = sumsq/n
                    eng.tensor_scalar_mul(out=st2, in0=sums, scalar1=1.0 / N)
                # group means of (mean, E[x^2]) via indicator matmul
                pg = psB.tile([4, 2], F32, name="psg", tag="pav")
                nc.tensor.matmul(pg, ind1_s, st2, start=True, stop=True)
                pgs = sp.tile([4, 2], F32, name="gnpgs")
                eng.tensor_copy(out=pgs, in_=pg)
                gm2 = sp.tile([4, 1], F32, name="gngm2")
                eng.tensor_mul(out=gm2, in0=pgs[:, 0:1], in1=pgs[:, 0:1])
                gvar = sp.tile([4, 1], F32, name="gnvar")
                eng.tensor_tensor(out=gvar, in0=pgs[:, 1:2], in1=gm2, op=OP.subtract)
                gstd = sp.tile([4, 1], F32, name="gnstd")
                nc.scalar.activation(out=gstd, in_=gvar, func=AF.Sqrt, bias=eps4, scale=1.0)
                grstd = sp.tile([4, 1], F32, name="gnrstd")
                nc.vector.reciprocal(out=grstd, in_=gstd)
                gvals = sp.tile([4, 2], F32R, name="gnvals")
                eng.tensor_copy(out=gvals[:, 0:1], in_=pgs[:, 0:1])
                eng.tensor_copy(out=gvals[:, 1:2], in_=grstd)
                # broadcast group (mean, rstd) back to channels
                pb = psB.tile([P, 2], F32, name="psb2", tag="pav")
                nc.tensor.matmul(pb, ind2_s, gvals, start=True, stop=True)
                s_v = sp.tile([P, 1], F32, name="gns")
                eng.tensor_mul(out=s_v, in0=pb[:, 1:2], in1=vecs[("gam", cc)])
                ms = sp.tile([P, 1], F32, name="gnms")
                eng.tensor_mul(out=ms, in0=pb[:, 0:1], in1=s_v)
                t_v = sp.tile([P, 1], F32, name="gnt")
                eng.tensor_tensor(out=t_v, in0=vecs[("bet", cc)], in1=ms, op=OP.subtract)
                s_vecs.append(s_v)
                t_vecs.append(t_v)

            # ---------- fold GN into weights; effective biases ----------
            wr = {}
            for nm in ("wq", "wk", "wv"):
                for cc in range(CC):
                    t = pp.tile([P, C], F32R, name=f"{nm}r{cc}")
                    nc.vector.tensor_scalar_mul(out=t, in0=wraw[(nm, cc)], scalar1=s_vecs[cc])
                    wr[(nm, cc)] = t
            for cc in range(CC):
                t = pp.tile([P, C], F32R, name=f"wpr{cc}")
                nc.vector.tensor_copy(out=t, in_=wraw[("wp", cc)])
                wr[("wp", cc)] = t

            beff = {}
            for nm in ("wq", "wk", "wv"):
                bnm = "b" + nm[1]
                for oc in range(CC):
                    pbx = psB.tile([P, 1], F32, name="psbias", tag="pav")
                    for cc in range(CC):
                        # raw (unfolded) weights: bias is w @ t, not (w*s) @ t.
                        # fp32 matmul is fine here (N=1).
                        nc.tensor.matmul(pbx, wraw[(nm, cc)][:, oc * P:(oc + 1) * P],
                                         t_vecs[cc], start=(cc == 0), stop=(cc == CC - 1))
                    t = pp.tile([P, 1], F32, name=f"beff_{nm}{oc}")
                    nc.scalar.activation(out=t, in_=pbx, func=AF.Identity,
                                         bias=vecs[(bnm, oc)], scale=1.0)
                    beff[(nm, oc)] = t
            # b* = wp @ bv_eff + bp (V bias folded through the projection)
            for oc in range(CC):
                pbx = psB.tile([P, 1], F32, name="psbias2", tag="pav")
                for cc in range(CC):
                    nc.tensor.matmul(pbx, wraw[("wp", cc)][:, oc * P:(oc + 1) * P],
                                     beff[("wv", cc)], start=(cc == 0), stop=(cc == CC - 1))
                t = pp.tile([P, 1], F32, name=f"bstar{oc}")
                nc.scalar.activation(out=t, in_=pbx, func=AF.Identity,
                                     bias=vecs[("bp", oc)], scale=1.0)
                beff[("wp", oc)] = t

            # ---------- projections ----------
            Qs = [pp.tile([P, NQ], F32R, name=f"Q{oc}") for oc in range(CC)]
            Ks = [pp.tile([P, N], F32R, name=f"K{oc}") for oc in range(CC)]
            for oc in range(CC):
                for i in range(NQ // QW):
                    pq = psA.tile([P, QW], F32, name="psq", tag="pst")
                    for h in range(2):
                        sl = slice(i * QW + h * 512, i * QW + (h + 1) * 512)
                        for cc in range(CC):
                            nc.tensor.matmul(pq[:, h * 512:(h + 1) * 512],
                                             wr[("wq", cc)][:, oc * P:(oc + 1) * P],
                                             xqs[cc][:, sl],
                                             start=(cc == 0), stop=(cc == CC - 1))
                    nc.scalar.activation(out=Qs[oc][:, i * QW:(i + 1) * QW], in_=pq,
                                         func=AF.Identity, bias=beff[("wq", oc)], scale=1.0)
                for i in range(N // QW):
                    pk = psA.tile([P, QW], F32, name="psk", tag="pst")
                    for h in range(2):
                        sl = slice(i * QW + h * 512, i * QW + (h + 1) * 512)
                        for cc in range(CC):
                            nc.tensor.matmul(pk[:, h * 512:(h + 1) * 512],
                                             wr[("wk", cc)][:, oc * P:(oc + 1) * P],
                                             xs[cc][:, sl],
                                             start=(cc == 0), stop=(cc == CC - 1))
                    nc.scalar.activation(out=Ks[oc][:, i * QW:(i + 1) * QW], in_=pk,
                                         func=AF.Identity, bias=beff[("wk", oc)], scale=1.0)
            VTs = pp.tile([P, NI * C], F32R, name="VTs")  # [128 tok, 32*256]
            for it in range(0, NI, 2):
                pv = psA.tile([P, 512], F32, name="psv", tag="pst")
                for j in range(2):
                    for cc in range(CC):
                        nc.tensor.matmul(pv[:, j * C:(j + 1) * C],
                                         xs[cc][:, (it + j) * P:(it + j + 1) * P],
                                         wr[("wv", cc)],
                                         start=(cc == 0), stop=(cc == CC - 1))
                nc.vector.tensor_copy(out=VTs[:, it * C:(it + 2) * C], in_=pv)

            # ---------- attention (QW=1024 query columns per pass) ----------
            for qp in range(NQP):
                pav = [psB.tile([P, QW], F32, name=f"pav{cc}", tag="pav") for cc in range(CC)]
                acc = etp.tile([P, QW], F32R, name="acc", tag="acc")
                accf = acc.bitcast(F32)
                acc_engs = [nc.vector, nc.gpsimd]
                for m in range(NI):
                    pst = psA.tile([P, QW], F32, name="pst", tag="pst")
                    for h in range(2):
                        for oc in range(CC):
                            nc.tensor.matmul(pst[:, h * 512:(h + 1) * 512],
                                             Ks[oc][:, m * P:(m + 1) * P],
                                             Qs[oc][:, qp * QW + h * 512:qp * QW + (h + 1) * 512],
                                             start=(oc == 0), stop=(oc == CC - 1))
                    et = etp.tile([P, QW], F32R, name="et", tag="et")
                    nc.scalar.activation(out=et, in_=pst, func=AF.Exp)
                    for h in range(2):
                        for cc in range(CC):
                            nc.tensor.matmul(pav[cc][:, h * 512:(h + 1) * 512],
                                             VTs[:, m * C + cc * P: m * C + (cc + 1) * P],
                                             et[:, h * 512:(h + 1) * 512],
                                             start=(m == 0), stop=(m == NI - 1))
                    for h in range(2):
                        sl = slice(h * 512, (h + 1) * 512)
                        if m == 0:
                            acc_engs[h].tensor_copy(out=acc[:, sl], in_=et[:, sl])
                        else:
                            acc_engs[h].tensor_tensor(out=acc[:, sl], in0=accf[:, sl],
                                                      in1=et.bitcast(F32)[:, sl], op=OP.add)
                # denominator -> broadcast reciprocal
                pd = psA.tile([P, QW], F32, name="psd", tag="pst")
                for h in range(2):
                    nc.tensor.matmul(pd[:, h * 512:(h + 1) * 512], allones_s,
                                     acc[:, h * 512:(h + 1) * 512], start=True, stop=True)
                rb = wkp.tile([P, QW], F32, name="rb", tag="rb")
                nc.vector.reciprocal_approx_fast(out=rb, in_=pd)
                obar = []
                for cc in range(CC):
                    ob = wkp.tile([P, QW], F32R, name="obar", tag="obar")
                    nc.vector.tensor_tensor(out=ob, in0=pav[cc], in1=rb, op=OP.mult)
                    obar.append(ob)
                for oc in range(CC):
                    py = psA.tile([P, QW], F32, name="psy", tag="pst")
                    for h in range(2):
                        for cc in range(CC):
                            nc.tensor.matmul(py[:, h * 512:(h + 1) * 512],
                                             wr[("wp", cc)][:, oc * P:(oc + 1) * P],
                                             obar[cc][:, h * 512:(h + 1) * 512],
                                             start=(cc == 0), stop=(cc == CC - 1))
                    y2 = wkp.tile([P, QW], F32, name="y2", tag="y2")
                    nc.vector.scalar_tensor_tensor(
                        out=y2, in0=py, scalar=beff[("wp", oc)],
                        in1=xqs[oc].bitcast(F32)[:, qp * QW:(qp + 1) * QW],
                        op0=OP.add, op1=OP.add)
                    nc.sync.dma_start(out=y_d[oc * P:(oc + 1) * P, qp * QW:(qp + 1) * QW],
                                      in_=y2)

    nc.finalize()
    return nc


def _get_nc():
    if "nc" not in _cache:
        _cache["nc"] = build_nc()
    return _cache["nc"]


def make_in_maps(x, gamma, beta, wq, bq, wk, bk, wv, bv, wp, bp):
    x = np.ascontiguousarray(np.asarray(x, dtype=np.float32))
    f32 = lambda a: np.ascontiguousarray(np.asarray(a, dtype=np.float32))
    scale = 1.0 / np.sqrt(np.float32(C))
    shared = {
        "wq_t": f32(np.asarray(wq, np.float32).T),
        "wk_t": f32(np.asarray(wk, np.float32).T * scale),
        "wv_t": f32(np.asarray(wv, np.float32).T),
        "wp_t": f32(np.asarray(wp, np.float32).T),
        "bq_v": f32(bq).reshape(C, 1),
        "bk_v": f32(np.asarray(bk, np.float32) * scale).reshape(C, 1),
        "bv_v": f32(bv).reshape(C, 1),
        "bp_v": f32(bp).reshape(C, 1),
        "gamma_v": f32(gamma).reshape(C, 1),
        "beta_v": f32(beta).reshape(C, 1),
    }
    in_maps = []
    for core in range(NCORES):
        bi, half = core // 2, core % 2
        x_r = round_tf32(x[bi].reshape(C, N))
        xq_r = np.ascontiguousarray(x_r[:, half * NQ:(half + 1) * NQ])
        m = dict(shared)
        m["x_r"] = x_r
        m["xq_r"] = xq_r
        in_maps.append(m)
    return in_maps


def run(inputs: dict, trace: bool = False):
    from concourse.bass_utils import run_bass_kernel_spmd
    nc = _get_nc()
    in_maps = make_in_maps(**inputs)
    res = run_bass_kernel_spmd(nc, in_maps, core_ids=list(range(NCORES)), trace=trace)
    y = np.empty((B, C, N), dtype=np.float32)
    for core in range(NCORES):
        bi, half = core // 2, core % 2
        y[bi][:, half * NQ:(half + 1) * NQ] = res.results[core]["y"]
    return y.reshape(B, C, H, W), res


def kernel(**inputs) -> np.ndarray:
    out, _ = run(inputs, trace=False)
    return out
